# revision 10
# baseline (speedup 1.0000x reference)
"""Bass/Trainium2 kernel for nn_BottomUpHTMM (bottom-up hidden tree Markov model).

Tree: complete 4-ary, depth 7, 21845 nodes. G=16 models, C=8 states, 256 labels.

v2 design:
- Each core owns 32 independent depth-3 subtrees rooted at the 256 level-4
  nodes (85..340): 32*(1+4+16+64) = 2720 nodes/core. Partition dim = (g,c).
- The prior chain of the reference cancels algebraically
  (prior*beta_il == A@beta_children), so only beta is propagated.
- Downward pass is factorized: eps(n) = eps(root_b) * D(n) where
  D(child) = D(parent)*bnr(parent)*m(child), bnr = 1/(A@beta_ch),
  m(child) = (W_l @ beta)(child). D is independent of anything above the
  subtree root, so each core emits per-root partial sums
  S_b = sum D*logb, S_A = sum Db*q, S_rho[l] = sum_{slot l} D, plus the root
  betas. No collective: the host computes the 341-node tree top and contracts
  eps(root) with the S terms.
- Emissions come from one ap_gather per buffer out of a host-built 1280-entry
  table (cols 0..255 = sm_b; cols 256+256*pos+label = leaf beta pre-normalized
  with sm_pi folded in). logb likewise (leaf entries include log pi).
- Reciprocals are computed as Exp(-Ln(x)) on the Scalar engine (DVE reciprocal
  is ~8 cycles/elem).

Per-core column layout: 32 blocks * 88 cols; block = [3 pad | root | 4 L1 |
16 L2 | 64 leaves]. Children of block-col c level j at co[j+1]+4*p+l,
co = [3, 4, 8, 24].
"""

import numpy as np
import ml_dtypes

BF16 = ml_dtypes.bfloat16

G = 16
C = 8
M = 256
L = 4
NCORES = 8
T_SIZE = 21845
NBLK = 32                 # subtrees per core
BLK = 88                  # cols per block (3 pad + 85 nodes)
NCOL = NBLK * BLK         # 2816
CO = [3, 4, 8, 24]        # level col offsets within block
O21 = [0, 1, 5]           # level offsets within the 21 internal slots
NB = [1, 4, 16]           # parents per block per level
NIDX = NCOL // 16         # 176

# out cols
OC_RB = 0     # 0..32   root beta
OC_SB = 32    # 32..64  S_b
OC_SA = 64    # 64..96  S_A
OC_RHO = 96   # 96..224 S_rho (block-major x l)
NOUT = 224


def _softmax(x, axis):
    e = np.exp(x - x.max(axis=axis, keepdims=True))
    return e / e.sum(axis=axis, keepdims=True)


def _wrap_idx(idx):
    """idx j at partition j%16, slot j//16, replicated across 8 gpsimd cores."""
    idx = np.asarray(idx, dtype=np.int16)
    n = len(idx)
    assert n % 16 == 0
    grid = idx.reshape(n // 16, 16).T          # [16, n/16]
    return np.tile(grid, (8, 1))               # [128, n/16]


def _block_cols():
    """Per-block arrays: node id builder + (col -> level, pos-slot)."""
    # block-relative heap ids per col (0 for pads), using cols 3..88
    rel = np.full(BLK, -1, np.int64)
    rel[3] = 0
    for p in range(21):                        # internal block-rel heap ids 0..20
        for l in range(4):
            rel[CO[1] + 4 * p + l if p == 0 else 0] = 0
    # simpler: levels explicitly
    rel[3] = 0
    rel[4:8] = np.arange(1, 5)
    rel[8:24] = np.arange(5, 21)
    rel[24:88] = np.arange(21, 85)
    return rel


_REL = _block_cols()


def _host_prep(t, a, b, pi, sp):
    t = np.asarray(t)
    labels = t[:, 0].astype(np.int64)
    a = np.asarray(a, np.float64)
    b = np.asarray(b, np.float64)
    pi = np.asarray(pi, np.float64)
    sp = np.asarray(sp, np.float64)
    sm_a = _softmax(a, 1)
    sm_b = _softmax(b, 2)
    sm_pi = _softmax(pi, 1)
    sm_sp = _softmax(sp, 1)
    asp = sm_a * sm_sp[:, None, None, :]

    # tables [128, 1280]
    tb = np.zeros((128, 1280), np.float32)
    tbl = np.zeros((128, 1280), np.float32)
    sb128 = sm_b.reshape(128, M)
    tb[:, :M] = sb128
    tbl[:, :M] = np.log(sb128)
    for pos in range(4):
        v = sm_b * sm_pi[:, :, pos][:, :, None]          # [G,C,M]
        s = v.sum(1, keepdims=True)
        tb[:, M + M * pos:M + M * (pos + 1)] = (v / s).reshape(128, M)
        tbl[:, M + M * pos:M + M * (pos + 1)] = np.log(v).reshape(128, M)

    # weights [128, 1152]: W0..3 V0..3 mbd
    la = np.log(sm_a)
    wv = np.zeros((128, 1024), np.float32)
    for l in range(4):
        Wl = np.zeros((128, 128))
        Vl = np.zeros((128, 128))
        for g in range(G):
            Wl[g * C:(g + 1) * C, g * C:(g + 1) * C] = asp[g, :, :, l].T      # [j,i]
            Vl[g * C:(g + 1) * C, g * C:(g + 1) * C] = (asp * la)[g, :, :, l].T
        wv[:, 128 * l:128 * (l + 1)] = Wl
        wv[:, 512 + 128 * l:512 + 128 * (l + 1)] = Vl

    wv = wv.astype(BF16)

    # per-core node ids + gather idx
    # block-rel heap: node 0 root, children of p at 4p+1+l
    gid_rel = np.zeros(85, np.int64)
    cores = []
    gids = []
    for k in range(NCORES):
        idx = np.zeros(NCOL, np.int64)
        gid_all = np.zeros((NBLK, 85), np.int64)
        for bq in range(NBLK):
            root = 85 + NBLK * k + bq
            gid_rel[0] = root
            for p in range(21):
                for l in range(4):
                    gid_rel[4 * p + 1 + l] = 4 * gid_rel[p] + 1 + l
            gid_all[bq] = gid_rel
            base = BLK * bq
            lab = labels[gid_rel]
            idx[base + 3:base + 24] = lab[:21]                     # internal: sm_b
            pos = (gid_rel[21:] - 1) % 4
            idx[base + 24:base + 88] = M + M * pos + lab[21:]      # leaves
        ebd = tb[:, idx].astype(np.float32)
        lgbd = tbl[:, idx].astype(np.float32)
        for bq in range(NBLK):
            ebd[:, BLK * bq:BLK * bq + 3] = 1.0
            lgbd[:, BLK * bq:BLK * bq + 3] = 0.0
        cores.append((ebd.astype(BF16), lgbd.astype(BF16)))
        gids.append(gid_all)

    host = dict(labels=labels, asp=asp, sm_b=sm_b, sm_pi=sm_pi,
                log_a=la, log_b=np.log(sm_b), log_sp=np.log(sm_sp))
    return tb, tbl, wv, cores, gids, host


def _combine(results, host):
    """Host: 341-node tree top + contraction with per-core S terms."""
    labels = host["labels"]; asp = host["asp"]; sm_b = host["sm_b"]
    log_a = host["log_a"]; log_b = host["log_b"]; log_sp = host["log_sp"]

    beta = np.zeros((341, G, C))
    Ab = np.zeros((85, G, C))
    for k in range(NCORES):
        rb = np.asarray(results[k]["out"], np.float64)
        rbm = rb[:, OC_RB:OC_RB + NBLK].T.reshape(NBLK, G, C)
        beta[85 + NBLK * k:85 + NBLK * (k + 1)] = \
            rbm / rbm.sum(2, keepdims=True)
    for lev in range(3, -1, -1):
        s, e = (4 ** lev - 1) // 3, (4 ** (lev + 1) - 1) // 3
        ch = 4 * np.arange(s, e)[:, None] + np.arange(1, 5)[None, :]
        AbP = np.einsum('gijl,plgj->pgi', asp, beta[ch])
        tmp = np.einsum('gcp,pgc->pgc', sm_b[:, :, labels[s:e]], AbP)
        beta[s:e] = tmp / tmp.sum(2, keepdims=True)
        Ab[s:e] = AbP
    eps = np.zeros((341, G, C)); eps[0] = beta[0]
    a_lh = np.zeros(G); rho = np.zeros((G, L))
    for lev in range(0, 4):
        s, e = (4 ** lev - 1) // 3, (4 ** (lev + 1) - 1) // 3
        ch = 4 * np.arange(s, e)[:, None] + np.arange(1, 5)[None, :]
        pe = eps[s:e] / Ab[s:e]
        mch = np.einsum('gijl,plgj->pgil', asp, beta[ch])
        epsc = pe[:, :, :, None] * mch
        for l in range(4):
            eps[ch[:, l]] = epsc[:, :, :, l]
        rho += epsc.sum(2).sum(0)
        a_lh += np.einsum('pgi,gijl,gijl,plgj->g', pe, asp, log_a, beta[ch])
    b_lh = np.einsum('ugc,gcu->g', eps, log_b[:, :, labels[:341]])

    # device terms
    for k in range(NCORES):
        out = np.asarray(results[k]["out"], np.float64)
        er = eps[85 + NBLK * k:85 + NBLK * (k + 1)].reshape(NBLK, 128)  # [b,(g,c)]
        S_b = out[:, OC_SB:OC_SB + NBLK].T          # [b, 128]
        S_A = out[:, OC_SA:OC_SA + NBLK].T
        S_r = out[:, OC_RHO:OC_RHO + 4 * NBLK].T.reshape(NBLK, 4, 128)
        b_lh += (er * S_b).reshape(NBLK, G, C).sum(0).sum(1)
        a_lh += (er * S_A).reshape(NBLK, G, C).sum(0).sum(1)
        rho += np.einsum('blp,bp->pl', S_r, er).reshape(G, C, L).sum(1)
    sp_lh = (rho * log_sp).sum(1)
    return (a_lh + b_lh + sp_lh).astype(np.float32)


def build_bass():
    import concourse.bacc as bacc
    import concourse.tile as tile
    import concourse.mybir as mybir
    from concourse import bass

    f32 = mybir.dt.float32
    bf16 = mybir.dt.bfloat16
    AF = mybir.ActivationFunctionType
    ALU = mybir.AluOpType
    AX = mybir.AxisListType

    nc = bacc.Bacc("TRN2", target_bir_lowering=False, debug=False,
                   num_devices=NCORES)

    eb_in = nc.dram_tensor("ebd", [128, NCOL], bf16, kind="ExternalInput").ap()
    lgb_in = nc.dram_tensor("lgbd", [128, NCOL], bf16, kind="ExternalInput").ap()
    wv_in = nc.dram_tensor("wv", [128, 1024], bf16, kind="ExternalInput").ap()
    o_out = nc.dram_tensor("out", [128, NOUT], f32, kind="ExternalOutput").ap()

    with tile.TileContext(nc) as tc:
        with tc.tile_pool(name="per", bufs=1) as per, \
             tc.tile_pool(name="wrk", bufs=2) as wrk, \
             tc.tile_pool(name="ps", bufs=2, space="PSUM") as ps, \
             tc.tile_pool(name="ps3", bufs=3, space="PSUM") as ps3:

            wv = per.tile([128, 1024], bf16, tag="wv")
            W = [wv[:, 128 * l:128 * (l + 1)] for l in range(4)]
            V = [wv[:, 512 + 128 * l:512 + 128 * (l + 1)] for l in range(4)]

            eb = per.tile([128, NCOL], bf16, tag="eb")      # emission -> beta
            lgb = per.tile([128, NCOL], bf16, tag="lgb")    # log emission
            nc.sync.dma_start(out=wv[:], in_=wv_in)
            nc.sync.dma_start(out=eb[:, :NCOL // 2], in_=eb_in[:, :NCOL // 2])
            nc.sync.dma_start(out=eb[:, NCOL // 2:], in_=eb_in[:, NCOL // 2:])
            nc.sync.dma_start(out=lgb[:], in_=lgb_in)
            Dt = per.tile([128, NCOL], bf16, tag="Dt")      # eps factor D
            sbp = per.tile([128, NCOL], bf16, tag="sbp")    # D*logb scratch
            bnr = per.tile([128, NBLK, 21], bf16, tag="bnr")
            SA = per.tile([128, NBLK, 21], f32, tag="SA")
            outp = per.tile([128, NOUT], f32, tag="outp")

            ebv = eb[:].rearrange("p (b c) -> p b c", b=NBLK)
            lgv = lgb[:].rearrange("p (b c) -> p b c", b=NBLK)
            Dv = Dt[:].rearrange("p (b c) -> p b c", b=NBLK)
            sbv = sbp[:].rearrange("p (b c) -> p b c", b=NBLK)

            def child_view(b0, nbl, j, l):
                """beta of l-th children of level-j parents: [128, nbl, NB[j]]"""
                v = ebv[:, b0:b0 + nbl, CO[j + 1]:CO[j + 1] + 4 * NB[j]]
                return v.rearrange("p b (n l) -> p b n l", l=4)[:, :, :, l]

            # ---------------- upward (no per-node normalization) ----------------
            for u, (j, b0, nbl) in enumerate([(2, 0, 16), (2, 16, 16),
                                              (1, 0, 32), (0, 0, 32)]):
                n_b = NB[j]
                ub = ps3.tile([128, nbl, n_b], f32, tag="ub")
                for l in range(4):
                    nc.tensor.matmul(ub[:], W[l], child_view(b0, nbl, j, l),
                                     start=(l == 0), stop=(l == 3))
                with nc.allow_low_precision(reason="bnr bf16 ok (tol 2e-2)"):
                    nc.vector.reciprocal(
                        bnr[:, b0:b0 + nbl, O21[j]:O21[j] + n_b], ub[:])
                nc.vector.tensor_tensor(
                    ebv[:, b0:b0 + nbl, CO[j]:CO[j] + n_b],
                    ebv[:, b0:b0 + nbl, CO[j]:CO[j] + n_b], ub[:], ALU.mult)

            # root (unnormalized) betas to output
            nc.scalar.copy(out=outp[:, OC_RB:OC_RB + NBLK], in_=ebv[:, :, 3])

            # ---------------- downward (D chain + S terms) ----------------
            Db0 = bnr[:, :, 0]                                   # [128, 32]
            # j = 0
            m0 = ps.tile([128, 4, NBLK], f32, tag="m")
            for l in range(4):
                nc.tensor.matmul(m0[:, l, :], W[l],
                                 child_view(0, NBLK, 0, l)
                                 .rearrange("p b n -> p (b n)"),
                                 start=True, stop=True)
            q0 = ps.tile([128, NBLK], f32, tag="q")
            for l in range(4):
                nc.tensor.matmul(q0[:], V[l],
                                 child_view(0, NBLK, 0, l)
                                 .rearrange("p b n -> p (b n)"),
                                 start=(l == 0), stop=(l == 3))
            nc.vector.tensor_tensor(Dv[:, :, 4:8], m0[:].transpose([0, 2, 1]),
                                    Db0[:, :, None].to_broadcast([128, NBLK, 4]),
                                    ALU.mult)
            nc.vector.tensor_tensor(SA[:, :, 0:1], Db0[:, :, None], q0[:, :, None],
                                    ALU.mult)
            # j = 1
            Db1 = wrk.tile([128, NBLK, 4], bf16, tag="db1")
            nc.vector.tensor_tensor(Db1[:], Dv[:, :, 4:8], bnr[:, :, 1:5], ALU.mult)
            m1 = ps.tile([128, 4, NBLK, 4], f32, tag="m")
            for l in range(4):
                nc.tensor.matmul(m1[:, l, :, :], W[l], child_view(0, NBLK, 1, l),
                                 start=True, stop=True)
            q1 = ps.tile([128, NBLK, 4], f32, tag="q")
            for l in range(4):
                nc.tensor.matmul(q1[:], V[l], child_view(0, NBLK, 1, l),
                                 start=(l == 0), stop=(l == 3))
            nc.vector.tensor_tensor(
                Dv[:, :, 8:24].rearrange("p b (n l) -> p b n l", l=4),
                m1[:].transpose([0, 2, 3, 1]),
                Db1[:, :, :, None].to_broadcast([128, NBLK, 4, 4]), ALU.mult)
            nc.vector.tensor_tensor(SA[:, :, 1:5], Db1[:], q1[:], ALU.mult)
            # j = 2
            Db2 = wrk.tile([128, NBLK, 16], bf16, tag="db2")
            nc.vector.tensor_tensor(Db2[:], Dv[:, :, 8:24], bnr[:, :, 5:21],
                                    ALU.mult)
            q2 = ps.tile([128, NBLK, 16], f32, tag="q")
            for l in range(4):
                nc.tensor.matmul(q2[:], V[l], child_view(0, NBLK, 2, l),
                                 start=(l == 0), stop=(l == 3))
            nc.vector.tensor_tensor(SA[:, :, 5:21], Db2[:], q2[:], ALU.mult)
            nc.vector.tensor_reduce(outp[:, OC_SA:OC_SA + NBLK],
                                    SA[:], axis=AX.X, op=ALU.add)
            for b0 in range(0, NBLK, 8):
                m2 = ps.tile([128, 4, 8, 16], f32, tag="m")
                for l in range(4):
                    nc.tensor.matmul(m2[:, l, :, :], W[l], child_view(b0, 8, 2, l),
                                     start=True, stop=True)
                nc.vector.tensor_tensor(
                    Dv[:, b0:b0 + 8, 24:88].rearrange("p b (n l) -> p b n l", l=4),
                    m2[:].transpose([0, 2, 3, 1]),
                    Db2[:, b0:b0 + 8, :, None].to_broadcast([128, 8, 16, 4]),
                    ALU.mult)
                # per-chunk endgame: S_b product + reductions
                nc.vector.tensor_tensor(sbv[:, b0:b0 + 8, 4:88],
                                        Dv[:, b0:b0 + 8, 4:88],
                                        lgv[:, b0:b0 + 8, 4:88], ALU.mult)
                nc.vector.tensor_reduce(outp[:, OC_SB + b0:OC_SB + b0 + 8],
                                        sbv[:, b0:b0 + 8, 4:88],
                                        axis=AX.X, op=ALU.add)
                rhov = Dv[:, b0:b0 + 8, 4:88].rearrange("p b (n l) -> p b l n", l=4)
                nc.vector.tensor_reduce(
                    outp[:, OC_RHO + 4 * b0:OC_RHO + 4 * (b0 + 8)]
                    .rearrange("p (b l) -> p b l", l=4),
                    rhov, axis=AX.X, op=ALU.add)

            nc.sync.dma_start(out=o_out, in_=outp[:])

    nc.finalize()
    return nc


_NC_CACHE = {}


def _shard_inputs(t, a, b, pi, sp):
    tb, tbl, wv, cores, gids, host = _host_prep(t, a, b, pi, sp)
    in_maps = []
    for k in range(NCORES):
        in_maps.append({"ebd": cores[k][0], "lgbd": cores[k][1], "wv": wv})
    return in_maps, host


def kernel(t, t_limits, a, b, pi, sp):
    from concourse.bass_utils import run_bass_kernel_spmd
    if "nc" not in _NC_CACHE:
        _NC_CACHE["nc"] = build_bass()
    nc = _NC_CACHE["nc"]
    in_maps, host = _shard_inputs(t, a, b, pi, sp)
    res = run_bass_kernel_spmd(nc, in_maps, list(range(NCORES)))
    return _combine(res.results, host)


# revision 11
# speedup vs baseline: 1.0508x; 1.0508x over previous
"""Bass/Trainium2 kernel for nn_BottomUpHTMM (bottom-up hidden tree Markov model).

Tree: complete 4-ary, depth 7, 21845 nodes. G=16 models, C=8 states, 256 labels.

v2 design:
- Each core owns 32 independent depth-3 subtrees rooted at the 256 level-4
  nodes (85..340): 32*(1+4+16+64) = 2720 nodes/core. Partition dim = (g,c).
- The prior chain of the reference cancels algebraically
  (prior*beta_il == A@beta_children), so only beta is propagated.
- Downward pass is factorized: eps(n) = eps(root_b) * D(n) where
  D(child) = D(parent)*bnr(parent)*m(child), bnr = 1/(A@beta_ch),
  m(child) = (W_l @ beta)(child). D is independent of anything above the
  subtree root, so each core emits per-root partial sums
  S_b = sum D*logb, S_A = sum Db*q, S_rho[l] = sum_{slot l} D, plus the root
  betas. No collective: the host computes the 341-node tree top and contracts
  eps(root) with the S terms.
- Emissions come from one ap_gather per buffer out of a host-built 1280-entry
  table (cols 0..255 = sm_b; cols 256+256*pos+label = leaf beta pre-normalized
  with sm_pi folded in). logb likewise (leaf entries include log pi).
- Reciprocals are computed as Exp(-Ln(x)) on the Scalar engine (DVE reciprocal
  is ~8 cycles/elem).

Per-core column layout: 32 blocks * 88 cols; block = [3 pad | root | 4 L1 |
16 L2 | 64 leaves]. Children of block-col c level j at co[j+1]+4*p+l,
co = [3, 4, 8, 24].
"""

import numpy as np
import ml_dtypes

BF16 = ml_dtypes.bfloat16

G = 16
C = 8
M = 256
L = 4
NCORES = 8
T_SIZE = 21845
NBLK = 32                 # subtrees per core
BLK = 88                  # cols per block (3 pad + 85 nodes)
NCOL = NBLK * BLK         # 2816
CO = [3, 4, 8, 24]        # level col offsets within block
O21 = [0, 1, 5]           # level offsets within the 21 internal slots
NB = [1, 4, 16]           # parents per block per level
NIDX = NCOL // 16         # 176

# out cols
OC_RB = 0     # 0..32   root beta
OC_SB = 32    # 32..64  S_b
OC_SA = 64    # 64..96  S_A
OC_RHO = 96   # 96..224 S_rho (block-major x l)
NOUT = 224


def _softmax(x, axis):
    e = np.exp(x - x.max(axis=axis, keepdims=True))
    return e / e.sum(axis=axis, keepdims=True)


def _wrap_idx(idx):
    """idx j at partition j%16, slot j//16, replicated across 8 gpsimd cores."""
    idx = np.asarray(idx, dtype=np.int16)
    n = len(idx)
    assert n % 16 == 0
    grid = idx.reshape(n // 16, 16).T          # [16, n/16]
    return np.tile(grid, (8, 1))               # [128, n/16]


def _block_cols():
    """Per-block arrays: node id builder + (col -> level, pos-slot)."""
    # block-relative heap ids per col (0 for pads), using cols 3..88
    rel = np.full(BLK, -1, np.int64)
    rel[3] = 0
    for p in range(21):                        # internal block-rel heap ids 0..20
        for l in range(4):
            rel[CO[1] + 4 * p + l if p == 0 else 0] = 0
    # simpler: levels explicitly
    rel[3] = 0
    rel[4:8] = np.arange(1, 5)
    rel[8:24] = np.arange(5, 21)
    rel[24:88] = np.arange(21, 85)
    return rel


_REL = _block_cols()


def _host_prep(t, a, b, pi, sp):
    t = np.asarray(t)
    labels = t[:, 0].astype(np.int64)
    a = np.asarray(a, np.float64)
    b = np.asarray(b, np.float64)
    pi = np.asarray(pi, np.float64)
    sp = np.asarray(sp, np.float64)
    sm_a = _softmax(a, 1)
    sm_b = _softmax(b, 2)
    sm_pi = _softmax(pi, 1)
    sm_sp = _softmax(sp, 1)
    asp = sm_a * sm_sp[:, None, None, :]

    # tables [128, 1280]
    tb = np.zeros((128, 1280), np.float32)
    tbl = np.zeros((128, 1280), np.float32)
    sb128 = sm_b.reshape(128, M)
    tb[:, :M] = sb128
    tbl[:, :M] = np.log(sb128)
    for pos in range(4):
        v = sm_b * sm_pi[:, :, pos][:, :, None]          # [G,C,M]
        s = v.sum(1, keepdims=True)
        tb[:, M + M * pos:M + M * (pos + 1)] = (v / s).reshape(128, M)
        tbl[:, M + M * pos:M + M * (pos + 1)] = np.log(v).reshape(128, M)

    # weights [128, 1152]: W0..3 V0..3 mbd
    la = np.log(sm_a)
    wv = np.zeros((128, 1024), np.float32)
    for l in range(4):
        Wl = np.zeros((128, 128))
        Vl = np.zeros((128, 128))
        for g in range(G):
            Wl[g * C:(g + 1) * C, g * C:(g + 1) * C] = asp[g, :, :, l].T      # [j,i]
            Vl[g * C:(g + 1) * C, g * C:(g + 1) * C] = (asp * la)[g, :, :, l].T
        wv[:, 128 * l:128 * (l + 1)] = Wl
        wv[:, 512 + 128 * l:512 + 128 * (l + 1)] = Vl

    wv = wv.astype(BF16)

    # per-core node ids + gather idx
    # block-rel heap: node 0 root, children of p at 4p+1+l
    gid_rel = np.zeros(85, np.int64)
    cores = []
    gids = []
    for k in range(NCORES):
        idx = np.zeros(NCOL, np.int64)
        gid_all = np.zeros((NBLK, 85), np.int64)
        for bq in range(NBLK):
            root = 85 + NBLK * k + bq
            gid_rel[0] = root
            for p in range(21):
                for l in range(4):
                    gid_rel[4 * p + 1 + l] = 4 * gid_rel[p] + 1 + l
            gid_all[bq] = gid_rel
            base = BLK * bq
            lab = labels[gid_rel]
            idx[base + 3:base + 24] = lab[:21]                     # internal: sm_b
            pos = (gid_rel[21:] - 1) % 4
            idx[base + 24:base + 88] = M + M * pos + lab[21:]      # leaves
        ebd = tb[:, idx].astype(np.float32)
        lgbd = tbl[:, idx].astype(np.float32)
        for bq in range(NBLK):
            ebd[:, BLK * bq:BLK * bq + 3] = 1.0
            lgbd[:, BLK * bq:BLK * bq + 3] = 0.0
        cores.append((ebd.astype(BF16), lgbd.astype(BF16)))
        gids.append(gid_all)

    host = dict(labels=labels, asp=asp, sm_b=sm_b, sm_pi=sm_pi,
                log_a=la, log_b=np.log(sm_b), log_sp=np.log(sm_sp))
    return tb, tbl, wv, cores, gids, host


def _combine(results, host):
    """Host: 341-node tree top + contraction with per-core S terms."""
    labels = host["labels"]; asp = host["asp"]; sm_b = host["sm_b"]
    log_a = host["log_a"]; log_b = host["log_b"]; log_sp = host["log_sp"]

    beta = np.zeros((341, G, C))
    Ab = np.zeros((85, G, C))
    for k in range(NCORES):
        rb = np.asarray(results[k]["out"], np.float64)
        rbm = rb[:, OC_RB:OC_RB + NBLK].T.reshape(NBLK, G, C)
        beta[85 + NBLK * k:85 + NBLK * (k + 1)] = \
            rbm / rbm.sum(2, keepdims=True)
    for lev in range(3, -1, -1):
        s, e = (4 ** lev - 1) // 3, (4 ** (lev + 1) - 1) // 3
        ch = 4 * np.arange(s, e)[:, None] + np.arange(1, 5)[None, :]
        AbP = np.einsum('gijl,plgj->pgi', asp, beta[ch])
        tmp = np.einsum('gcp,pgc->pgc', sm_b[:, :, labels[s:e]], AbP)
        beta[s:e] = tmp / tmp.sum(2, keepdims=True)
        Ab[s:e] = AbP
    eps = np.zeros((341, G, C)); eps[0] = beta[0]
    a_lh = np.zeros(G); rho = np.zeros((G, L))
    for lev in range(0, 4):
        s, e = (4 ** lev - 1) // 3, (4 ** (lev + 1) - 1) // 3
        ch = 4 * np.arange(s, e)[:, None] + np.arange(1, 5)[None, :]
        pe = eps[s:e] / Ab[s:e]
        mch = np.einsum('gijl,plgj->pgil', asp, beta[ch])
        epsc = pe[:, :, :, None] * mch
        for l in range(4):
            eps[ch[:, l]] = epsc[:, :, :, l]
        rho += epsc.sum(2).sum(0)
        a_lh += np.einsum('pgi,gijl,gijl,plgj->g', pe, asp, log_a, beta[ch])
    b_lh = np.einsum('ugc,gcu->g', eps, log_b[:, :, labels[:341]])

    # device terms
    for k in range(NCORES):
        out = np.asarray(results[k]["out"], np.float64)
        er = eps[85 + NBLK * k:85 + NBLK * (k + 1)].reshape(NBLK, 128)  # [b,(g,c)]
        S_b = out[:, OC_SB:OC_SB + NBLK].T          # [b, 128]
        S_A = out[:, OC_SA:OC_SA + NBLK].T
        S_r = out[:, OC_RHO:OC_RHO + 4 * NBLK].T.reshape(NBLK, 4, 128)
        b_lh += (er * S_b).reshape(NBLK, G, C).sum(0).sum(1)
        a_lh += (er * S_A).reshape(NBLK, G, C).sum(0).sum(1)
        rho += np.einsum('blp,bp->pl', S_r, er).reshape(G, C, L).sum(1)
    sp_lh = (rho * log_sp).sum(1)
    return (a_lh + b_lh + sp_lh).astype(np.float32)


def build_bass():
    import concourse.bacc as bacc
    import concourse.tile as tile
    import concourse.mybir as mybir
    from concourse import bass

    f32 = mybir.dt.float32
    bf16 = mybir.dt.bfloat16
    AF = mybir.ActivationFunctionType
    ALU = mybir.AluOpType
    AX = mybir.AxisListType

    nc = bacc.Bacc("TRN2", target_bir_lowering=False, debug=False,
                   num_devices=NCORES)

    eb_in = nc.dram_tensor("ebd", [128, NCOL], bf16, kind="ExternalInput").ap()
    lgb_in = nc.dram_tensor("lgbd", [128, NCOL], bf16, kind="ExternalInput").ap()
    wv_in = nc.dram_tensor("wv", [128, 1024], bf16, kind="ExternalInput").ap()
    o_out = nc.dram_tensor("out", [128, NOUT], f32, kind="ExternalOutput").ap()

    with tile.TileContext(nc) as tc:
        with tc.tile_pool(name="per", bufs=1) as per, \
             tc.tile_pool(name="wrk", bufs=2) as wrk, \
             tc.tile_pool(name="ps", bufs=2, space="PSUM") as ps, \
             tc.tile_pool(name="ps3", bufs=3, space="PSUM") as ps3:

            wv = per.tile([128, 1024], bf16, tag="wv")
            W = [wv[:, 128 * l:128 * (l + 1)] for l in range(4)]
            V = [wv[:, 512 + 128 * l:512 + 128 * (l + 1)] for l in range(4)]

            eb = per.tile([128, NCOL], bf16, tag="eb")      # emission -> beta
            lgb = per.tile([128, NCOL], bf16, tag="lgb")    # log emission
            nc.sync.dma_start(out=wv[:], in_=wv_in)
            nc.scalar.dma_start(out=eb[:, :NCOL // 2], in_=eb_in[:, :NCOL // 2])
            nc.scalar.dma_start(out=eb[:, NCOL // 2:], in_=eb_in[:, NCOL // 2:])
            nc.sync.dma_start(out=lgb[:], in_=lgb_in)
            Dt = per.tile([128, NCOL], bf16, tag="Dt")      # eps factor D
            sbp = per.tile([128, NCOL], bf16, tag="sbp")    # D*logb scratch
            bnr = per.tile([128, NBLK, 21], bf16, tag="bnr")
            SA = per.tile([128, NBLK, 21], f32, tag="SA")
            outp = per.tile([128, NOUT], f32, tag="outp")
            acsc = per.tile([128, 84], f32, tag="acsc")

            ebv = eb[:].rearrange("p (b c) -> p b c", b=NBLK)
            lgv = lgb[:].rearrange("p (b c) -> p b c", b=NBLK)
            Dv = Dt[:].rearrange("p (b c) -> p b c", b=NBLK)
            sbv = sbp[:].rearrange("p (b c) -> p b c", b=NBLK)

            def child_view(b0, nbl, j, l):
                """beta of l-th children of level-j parents: [128, nbl, NB[j]]"""
                v = ebv[:, b0:b0 + nbl, CO[j + 1]:CO[j + 1] + 4 * NB[j]]
                return v.rearrange("p b (n l) -> p b n l", l=4)[:, :, :, l]

            # ---------------- upward (no per-node normalization) ----------------
            lns = []
            for u, (j, b0, nbl) in enumerate([(2, 0, 16), (2, 16, 16),
                                              (1, 0, 32), (0, 0, 32)]):
                n_b = NB[j]
                ub = ps3.tile([128, nbl, n_b], f32, tag="ub")
                for l in range(4):
                    nc.tensor.matmul(ub[:], W[l], child_view(b0, nbl, j, l),
                                     start=(l == 0), stop=(l == 3))
                if j == 2:
                    ln = per.tile([128, nbl, n_b], f32, tag=f"ln{u}")
                    nc.scalar.activation(out=ln[:], in_=ub[:], func=AF.Ln)
                    lns.append((j, b0, nbl, ln))
                else:
                    with nc.allow_low_precision(reason="bnr bf16 ok (tol 2e-2)"):
                        nc.vector.reciprocal(
                            bnr[:, b0:b0 + nbl, O21[j]:O21[j] + n_b], ub[:])
                nc.vector.tensor_tensor(
                    ebv[:, b0:b0 + nbl, CO[j]:CO[j] + n_b],
                    ebv[:, b0:b0 + nbl, CO[j]:CO[j] + n_b], ub[:], ALU.mult)
            for j, b0, nbl, ln in lns:
                nc.scalar.activation(out=bnr[:, b0:b0 + nbl, O21[j]:O21[j] + NB[j]],
                                     in_=ln[:], func=AF.Exp, scale=-1.0)

            # root (unnormalized) betas to output
            nc.scalar.copy(out=outp[:, OC_RB:OC_RB + NBLK], in_=ebv[:, :, 3])

            # ---------------- downward (D chain + S terms) ----------------
            Db0 = bnr[:, :, 0]                                   # [128, 32]
            # j = 0
            m0 = ps.tile([128, 4, NBLK], f32, tag="m")
            for l in range(4):
                nc.tensor.matmul(m0[:, l, :], W[l],
                                 child_view(0, NBLK, 0, l)
                                 .rearrange("p b n -> p (b n)"),
                                 start=True, stop=True)
            q0 = ps.tile([128, NBLK], f32, tag="q")
            for l in range(4):
                nc.tensor.matmul(q0[:], V[l],
                                 child_view(0, NBLK, 0, l)
                                 .rearrange("p b n -> p (b n)"),
                                 start=(l == 0), stop=(l == 3))
            nc.vector.tensor_tensor(Dv[:, :, 4:8], m0[:].transpose([0, 2, 1]),
                                    Db0[:, :, None].to_broadcast([128, NBLK, 4]),
                                    ALU.mult)
            nc.vector.tensor_tensor(SA[:, :, 0:1], Db0[:, :, None], q0[:, :, None],
                                    ALU.mult)
            # j = 1
            Db1 = wrk.tile([128, NBLK, 4], bf16, tag="db1")
            nc.vector.tensor_tensor(Db1[:], Dv[:, :, 4:8], bnr[:, :, 1:5], ALU.mult)
            m1 = ps.tile([128, 4, NBLK, 4], f32, tag="m")
            for l in range(4):
                nc.tensor.matmul(m1[:, l, :, :], W[l], child_view(0, NBLK, 1, l),
                                 start=True, stop=True)
            q1 = ps.tile([128, NBLK, 4], f32, tag="q")
            for l in range(4):
                nc.tensor.matmul(q1[:], V[l], child_view(0, NBLK, 1, l),
                                 start=(l == 0), stop=(l == 3))
            nc.vector.tensor_tensor(
                Dv[:, :, 8:24].rearrange("p b (n l) -> p b n l", l=4),
                m1[:].transpose([0, 2, 3, 1]),
                Db1[:, :, :, None].to_broadcast([128, NBLK, 4, 4]), ALU.mult)
            nc.vector.tensor_tensor(SA[:, :, 1:5], Db1[:], q1[:], ALU.mult)
            # j = 2
            Db2 = wrk.tile([128, NBLK, 16], bf16, tag="db2")
            nc.vector.tensor_tensor(Db2[:], Dv[:, :, 8:24], bnr[:, :, 5:21],
                                    ALU.mult)
            q2 = ps.tile([128, NBLK, 16], f32, tag="q")
            for l in range(4):
                nc.tensor.matmul(q2[:], V[l], child_view(0, NBLK, 2, l),
                                 start=(l == 0), stop=(l == 3))
            nc.vector.tensor_tensor(SA[:, :, 5:21], Db2[:], q2[:], ALU.mult)
            nc.vector.tensor_reduce(outp[:, OC_SA:OC_SA + NBLK],
                                    SA[:], axis=AX.X, op=ALU.add)
            for b0 in range(0, NBLK, 8):
                m2 = ps.tile([128, 4, 8, 16], f32, tag="m")
                for l in range(4):
                    nc.tensor.matmul(m2[:, l, :, :], W[l], child_view(b0, 8, 2, l),
                                     start=True, stop=True)
                nc.vector.tensor_tensor(
                    Dv[:, b0:b0 + 8, 24:88].rearrange("p b (n l) -> p b n l", l=4),
                    m2[:].transpose([0, 2, 3, 1]),
                    Db2[:, b0:b0 + 8, :, None].to_broadcast([128, 8, 16, 4]),
                    ALU.mult)
                # per-chunk endgame: S_b product + reductions
                nc.gpsimd.tensor_tensor(sbv[:, b0:b0 + 8, 4:88],
                                        Dv[:, b0:b0 + 8, 4:88],
                                        lgv[:, b0:b0 + 8, 4:88], ALU.mult)
                if b0 < 16:
                    nc.vector.tensor_reduce(outp[:, OC_SB + b0:OC_SB + b0 + 8],
                                            sbv[:, b0:b0 + 8, 4:88],
                                            axis=AX.X, op=ALU.add)
                else:
                    for bq in range(b0, b0 + 8):
                        nc.scalar.activation(
                            out=acsc[:], in_=sbv[:, bq, 4:88], func=AF.Copy,
                            accum_out=outp[:, OC_SB + bq:OC_SB + bq + 1])
                rhov = Dv[:, b0:b0 + 8, 4:88].rearrange("p b (n l) -> p b l n", l=4)
                nc.vector.tensor_reduce(
                    outp[:, OC_RHO + 4 * b0:OC_RHO + 4 * (b0 + 8)]
                    .rearrange("p (b l) -> p b l", l=4),
                    rhov, axis=AX.X, op=ALU.add)

            nc.sync.dma_start(out=o_out, in_=outp[:])

    nc.finalize()
    return nc


_NC_CACHE = {}


def _shard_inputs(t, a, b, pi, sp):
    tb, tbl, wv, cores, gids, host = _host_prep(t, a, b, pi, sp)
    in_maps = []
    for k in range(NCORES):
        in_maps.append({"ebd": cores[k][0], "lgbd": cores[k][1], "wv": wv})
    return in_maps, host


def kernel(t, t_limits, a, b, pi, sp):
    from concourse.bass_utils import run_bass_kernel_spmd
    if "nc" not in _NC_CACHE:
        _NC_CACHE["nc"] = build_bass()
    nc = _NC_CACHE["nc"]
    in_maps, host = _shard_inputs(t, a, b, pi, sp)
    res = run_bass_kernel_spmd(nc, in_maps, list(range(NCORES)))
    return _combine(res.results, host)


# revision 12
# speedup vs baseline: 1.1620x; 1.1058x over previous
"""Bass/Trainium2 kernel for nn_BottomUpHTMM (bottom-up hidden tree Markov model).

Tree: complete 4-ary, depth 7, 21845 nodes. G=16 models, C=8 states, 256 labels.

v2 design:
- Each core owns 32 independent depth-3 subtrees rooted at the 256 level-4
  nodes (85..340): 32*(1+4+16+64) = 2720 nodes/core. Partition dim = (g,c).
- The prior chain of the reference cancels algebraically
  (prior*beta_il == A@beta_children), so only beta is propagated.
- Downward pass is factorized: eps(n) = eps(root_b) * D(n) where
  D(child) = D(parent)*bnr(parent)*m(child), bnr = 1/(A@beta_ch),
  m(child) = (W_l @ beta)(child). D is independent of anything above the
  subtree root, so each core emits per-root partial sums
  S_b = sum D*logb, S_A = sum Db*q, S_rho[l] = sum_{slot l} D, plus the root
  betas. No collective: the host computes the 341-node tree top and contracts
  eps(root) with the S terms.
- Emissions come from one ap_gather per buffer out of a host-built 1280-entry
  table (cols 0..255 = sm_b; cols 256+256*pos+label = leaf beta pre-normalized
  with sm_pi folded in). logb likewise (leaf entries include log pi).
- Reciprocals are computed as Exp(-Ln(x)) on the Scalar engine (DVE reciprocal
  is ~8 cycles/elem).

Per-core column layout: 32 blocks * 88 cols; block = [3 pad | root | 4 L1 |
16 L2 | 64 leaves]. Children of block-col c level j at co[j+1]+4*p+l,
co = [3, 4, 8, 24].
"""

import numpy as np
import ml_dtypes

BF16 = ml_dtypes.bfloat16

G = 16
C = 8
M = 256
L = 4
NCORES = 8
T_SIZE = 21845
NBLK = 32                 # subtrees per core
BLK = 88                  # cols per block (3 pad + 85 nodes)
NCOL = NBLK * BLK         # 2816
CO = [3, 4, 8, 24]        # level col offsets within block
O21 = [0, 1, 5]           # level offsets within the 21 internal slots
NB = [1, 4, 16]           # parents per block per level
NIDX = NCOL // 16         # 176

# out cols
OC_RB = 0     # 0..32   root beta
OC_SB = 32    # 32..64  S_b
OC_SA = 64    # 64..96  S_A
OC_RHO = 96   # 96..224 S_rho (block-major x l)
NOUT = 224


def _softmax(x, axis):
    e = np.exp(x - x.max(axis=axis, keepdims=True))
    return e / e.sum(axis=axis, keepdims=True)


def _wrap_idx(idx):
    """idx j at partition j%16, slot j//16, replicated across 8 gpsimd cores."""
    idx = np.asarray(idx, dtype=np.int16)
    n = len(idx)
    assert n % 16 == 0
    grid = idx.reshape(n // 16, 16).T          # [16, n/16]
    return np.tile(grid, (8, 1))               # [128, n/16]


def _block_cols():
    """Per-block arrays: node id builder + (col -> level, pos-slot)."""
    # block-relative heap ids per col (0 for pads), using cols 3..88
    rel = np.full(BLK, -1, np.int64)
    rel[3] = 0
    for p in range(21):                        # internal block-rel heap ids 0..20
        for l in range(4):
            rel[CO[1] + 4 * p + l if p == 0 else 0] = 0
    # simpler: levels explicitly
    rel[3] = 0
    rel[4:8] = np.arange(1, 5)
    rel[8:24] = np.arange(5, 21)
    rel[24:88] = np.arange(21, 85)
    return rel


_REL = _block_cols()


def _host_prep(t, a, b, pi, sp):
    t = np.asarray(t)
    labels = t[:, 0].astype(np.int64)
    a = np.asarray(a, np.float64)
    b = np.asarray(b, np.float64)
    pi = np.asarray(pi, np.float64)
    sp = np.asarray(sp, np.float64)
    sm_a = _softmax(a, 1)
    sm_b = _softmax(b, 2)
    sm_pi = _softmax(pi, 1)
    sm_sp = _softmax(sp, 1)
    asp = sm_a * sm_sp[:, None, None, :]

    # tables [128, 1280]
    tb = np.zeros((128, 1280), np.float32)
    tbl = np.zeros((128, 1280), np.float32)
    sb128 = sm_b.reshape(128, M)
    tb[:, :M] = sb128
    tbl[:, :M] = np.log(sb128)
    for pos in range(4):
        v = sm_b * sm_pi[:, :, pos][:, :, None]          # [G,C,M]
        s = v.sum(1, keepdims=True)
        tb[:, M + M * pos:M + M * (pos + 1)] = (v / s).reshape(128, M)
        tbl[:, M + M * pos:M + M * (pos + 1)] = np.log(v).reshape(128, M)

    # weights [128, 1152]: W0..3 V0..3 mbd
    la = np.log(sm_a)
    wv = np.zeros((128, 1024), np.float32)
    for l in range(4):
        Wl = np.zeros((128, 128))
        Vl = np.zeros((128, 128))
        for g in range(G):
            Wl[g * C:(g + 1) * C, g * C:(g + 1) * C] = asp[g, :, :, l].T      # [j,i]
            Vl[g * C:(g + 1) * C, g * C:(g + 1) * C] = (asp * la)[g, :, :, l].T
        wv[:, 128 * l:128 * (l + 1)] = Wl
        wv[:, 512 + 128 * l:512 + 128 * (l + 1)] = Vl

    wv = wv.astype(BF16)

    # per-core node ids + gather idx
    # block-rel heap: node 0 root, children of p at 4p+1+l
    gid_rel = np.zeros(85, np.int64)
    cores = []
    gids = []
    for k in range(NCORES):
        idx = np.zeros(NCOL, np.int64)
        gid_all = np.zeros((NBLK, 85), np.int64)
        for bq in range(NBLK):
            root = 85 + NBLK * k + bq
            gid_rel[0] = root
            for p in range(21):
                for l in range(4):
                    gid_rel[4 * p + 1 + l] = 4 * gid_rel[p] + 1 + l
            gid_all[bq] = gid_rel
            base = BLK * bq
            lab = labels[gid_rel]
            idx[base + 3:base + 24] = lab[:21]                     # internal: sm_b
            pos = (gid_rel[21:] - 1) % 4
            idx[base + 24:base + 88] = M + M * pos + lab[21:]      # leaves
        ebd = tb[:, idx].astype(np.float32)
        lgbd = tbl[:, idx].astype(np.float32)
        for bq in range(NBLK):
            ebd[:, BLK * bq:BLK * bq + 3] = 1.0
            lgbd[:, BLK * bq:BLK * bq + 3] = 0.0
        cores.append((ebd.astype(BF16), lgbd.astype(BF16)))
        gids.append(gid_all)

    host = dict(labels=labels, asp=asp, sm_b=sm_b, sm_pi=sm_pi,
                log_a=la, log_b=np.log(sm_b), log_sp=np.log(sm_sp))
    return tb, tbl, wv, cores, gids, host


def _combine(results, host):
    """Host: 341-node tree top + contraction with per-core S terms."""
    labels = host["labels"]; asp = host["asp"]; sm_b = host["sm_b"]
    log_a = host["log_a"]; log_b = host["log_b"]; log_sp = host["log_sp"]

    beta = np.zeros((341, G, C))
    Ab = np.zeros((85, G, C))
    for k in range(NCORES):
        rb = np.asarray(results[k]["out"], np.float64)
        rbm = rb[:, OC_RB:OC_RB + NBLK].T.reshape(NBLK, G, C)
        beta[85 + NBLK * k:85 + NBLK * (k + 1)] = \
            rbm / rbm.sum(2, keepdims=True)
    for lev in range(3, -1, -1):
        s, e = (4 ** lev - 1) // 3, (4 ** (lev + 1) - 1) // 3
        ch = 4 * np.arange(s, e)[:, None] + np.arange(1, 5)[None, :]
        AbP = np.einsum('gijl,plgj->pgi', asp, beta[ch])
        tmp = np.einsum('gcp,pgc->pgc', sm_b[:, :, labels[s:e]], AbP)
        beta[s:e] = tmp / tmp.sum(2, keepdims=True)
        Ab[s:e] = AbP
    eps = np.zeros((341, G, C)); eps[0] = beta[0]
    a_lh = np.zeros(G); rho = np.zeros((G, L))
    for lev in range(0, 4):
        s, e = (4 ** lev - 1) // 3, (4 ** (lev + 1) - 1) // 3
        ch = 4 * np.arange(s, e)[:, None] + np.arange(1, 5)[None, :]
        pe = eps[s:e] / Ab[s:e]
        mch = np.einsum('gijl,plgj->pgil', asp, beta[ch])
        epsc = pe[:, :, :, None] * mch
        for l in range(4):
            eps[ch[:, l]] = epsc[:, :, :, l]
        rho += epsc.sum(2).sum(0)
        a_lh += np.einsum('pgi,gijl,gijl,plgj->g', pe, asp, log_a, beta[ch])
    b_lh = np.einsum('ugc,gcu->g', eps, log_b[:, :, labels[:341]])

    # device terms
    for k in range(NCORES):
        out = np.asarray(results[k]["out"], np.float64)
        er = eps[85 + NBLK * k:85 + NBLK * (k + 1)].reshape(NBLK, 128)  # [b,(g,c)]
        S_b = out[:, OC_SB:OC_SB + NBLK].T          # [b, 128]
        S_A = out[:, OC_SA:OC_SA + NBLK].T
        S_r = out[:, OC_RHO:OC_RHO + 4 * NBLK].T.reshape(NBLK, 4, 128)
        b_lh += (er * S_b).reshape(NBLK, G, C).sum(0).sum(1)
        a_lh += (er * S_A).reshape(NBLK, G, C).sum(0).sum(1)
        rho += np.einsum('blp,bp->pl', S_r, er).reshape(G, C, L).sum(1)
    sp_lh = (rho * log_sp).sum(1)
    return (a_lh + b_lh + sp_lh).astype(np.float32)


def build_bass():
    import concourse.bacc as bacc
    import concourse.tile as tile
    import concourse.mybir as mybir
    from concourse import bass

    f32 = mybir.dt.float32
    bf16 = mybir.dt.bfloat16
    AF = mybir.ActivationFunctionType
    ALU = mybir.AluOpType
    AX = mybir.AxisListType

    nc = bacc.Bacc("TRN2", target_bir_lowering=False, debug=False,
                   num_devices=NCORES)

    eb_in = nc.dram_tensor("ebd", [128, NCOL], bf16, kind="ExternalInput").ap()
    lgb_in = nc.dram_tensor("lgbd", [128, NCOL], bf16, kind="ExternalInput").ap()
    wv_in = nc.dram_tensor("wv", [128, 1024], bf16, kind="ExternalInput").ap()
    o_out = nc.dram_tensor("out", [128, NOUT], f32, kind="ExternalOutput").ap()

    with tile.TileContext(nc) as tc:
        with tc.tile_pool(name="per", bufs=1) as per, \
             tc.tile_pool(name="wrk", bufs=2) as wrk, \
             tc.tile_pool(name="ps", bufs=2, space="PSUM") as ps, \
             tc.tile_pool(name="ps3", bufs=3, space="PSUM") as ps3:

            wv = per.tile([128, 1024], bf16, tag="wv")
            W = [wv[:, 128 * l:128 * (l + 1)] for l in range(4)]
            V = [wv[:, 512 + 128 * l:512 + 128 * (l + 1)] for l in range(4)]

            eb = per.tile([128, NCOL], bf16, tag="eb")      # emission -> beta
            lgb = per.tile([128, NCOL], bf16, tag="lgb")    # log emission
            nc.sync.dma_start(out=wv[:], in_=wv_in)
            nc.scalar.dma_start(out=eb[:, :NCOL // 2], in_=eb_in[:, :NCOL // 2])
            nc.scalar.dma_start(out=eb[:, NCOL // 2:], in_=eb_in[:, NCOL // 2:])
            nc.sync.dma_start(out=lgb[:], in_=lgb_in)
            Dt = per.tile([128, NCOL], bf16, tag="Dt")      # eps factor D
            sbp = per.tile([128, NCOL], bf16, tag="sbp")    # D*logb scratch
            bnr = per.tile([128, NBLK, 21], bf16, tag="bnr")
            SA = per.tile([128, NBLK, 21], f32, tag="SA")
            outp = per.tile([128, NOUT], f32, tag="outp")

            ebv = eb[:].rearrange("p (b c) -> p b c", b=NBLK)
            lgv = lgb[:].rearrange("p (b c) -> p b c", b=NBLK)
            Dv = Dt[:].rearrange("p (b c) -> p b c", b=NBLK)
            sbv = sbp[:].rearrange("p (b c) -> p b c", b=NBLK)

            def child_view(b0, nbl, j, l):
                """beta of l-th children of level-j parents: [128, nbl, NB[j]]"""
                v = ebv[:, b0:b0 + nbl, CO[j + 1]:CO[j + 1] + 4 * NB[j]]
                return v.rearrange("p b (n l) -> p b n l", l=4)[:, :, :, l]

            # ---------------- upward (no per-node normalization) ----------------
            lns = []
            for u, (j, b0, nbl) in enumerate([(2, 0, 16), (2, 16, 16),
                                              (1, 0, 32), (0, 0, 32)]):
                n_b = NB[j]
                ub = ps3.tile([128, nbl, n_b], f32, tag="ub")
                for l in range(4):
                    nc.tensor.matmul(ub[:], W[l], child_view(b0, nbl, j, l),
                                     start=(l == 0), stop=(l == 3))
                if j == 2:
                    ln = per.tile([128, nbl, n_b], f32, tag=f"ln{u}")
                    nc.scalar.activation(out=ln[:], in_=ub[:], func=AF.Ln)
                    lns.append((j, b0, nbl, ln))
                else:
                    with nc.allow_low_precision(reason="bnr bf16 ok (tol 2e-2)"):
                        nc.vector.reciprocal(
                            bnr[:, b0:b0 + nbl, O21[j]:O21[j] + n_b], ub[:])
                nc.vector.tensor_tensor(
                    ebv[:, b0:b0 + nbl, CO[j]:CO[j] + n_b],
                    ebv[:, b0:b0 + nbl, CO[j]:CO[j] + n_b], ub[:], ALU.mult)
            for j, b0, nbl, ln in lns:
                nc.scalar.activation(out=bnr[:, b0:b0 + nbl, O21[j]:O21[j] + NB[j]],
                                     in_=ln[:], func=AF.Exp, scale=-1.0)

            # root (unnormalized) betas to output
            nc.scalar.copy(out=outp[:, OC_RB:OC_RB + NBLK], in_=ebv[:, :, 3])

            # ---------------- downward (D chain + S terms) ----------------
            Db0 = bnr[:, :, 0]                                   # [128, 32]
            # j = 0
            m0 = ps.tile([128, 4, NBLK], f32, tag="m")
            for l in range(4):
                nc.tensor.matmul(m0[:, l, :], W[l],
                                 child_view(0, NBLK, 0, l)
                                 .rearrange("p b n -> p (b n)"),
                                 start=True, stop=True)
            q0 = ps.tile([128, NBLK], f32, tag="q")
            for l in range(4):
                nc.tensor.matmul(q0[:], V[l],
                                 child_view(0, NBLK, 0, l)
                                 .rearrange("p b n -> p (b n)"),
                                 start=(l == 0), stop=(l == 3))
            nc.vector.tensor_tensor(Dv[:, :, 4:8], m0[:].transpose([0, 2, 1]),
                                    Db0[:, :, None].to_broadcast([128, NBLK, 4]),
                                    ALU.mult)
            nc.vector.tensor_tensor(SA[:, :, 0:1], Db0[:, :, None], q0[:, :, None],
                                    ALU.mult)
            # j = 1
            Db1 = wrk.tile([128, NBLK, 4], bf16, tag="db1")
            nc.vector.tensor_tensor(Db1[:], Dv[:, :, 4:8], bnr[:, :, 1:5], ALU.mult)
            m1 = ps.tile([128, 4, NBLK, 4], f32, tag="m")
            for l in range(4):
                nc.tensor.matmul(m1[:, l, :, :], W[l], child_view(0, NBLK, 1, l),
                                 start=True, stop=True)
            q1 = ps.tile([128, NBLK, 4], f32, tag="q")
            for l in range(4):
                nc.tensor.matmul(q1[:], V[l], child_view(0, NBLK, 1, l),
                                 start=(l == 0), stop=(l == 3))
            nc.vector.tensor_tensor(
                Dv[:, :, 8:24].rearrange("p b (n l) -> p b n l", l=4),
                m1[:].transpose([0, 2, 3, 1]),
                Db1[:, :, :, None].to_broadcast([128, NBLK, 4, 4]), ALU.mult)
            nc.vector.tensor_tensor(SA[:, :, 1:5], Db1[:], q1[:], ALU.mult)
            # j = 2
            Db2 = wrk.tile([128, NBLK, 16], bf16, tag="db2")
            nc.vector.tensor_tensor(Db2[:], Dv[:, :, 8:24], bnr[:, :, 5:21],
                                    ALU.mult)
            q2 = ps.tile([128, NBLK, 16], f32, tag="q")
            for l in range(4):
                nc.tensor.matmul(q2[:], V[l], child_view(0, NBLK, 2, l),
                                 start=(l == 0), stop=(l == 3))
            nc.vector.tensor_tensor(SA[:, :, 5:21], Db2[:], q2[:], ALU.mult)
            nc.vector.tensor_reduce(outp[:, OC_SA:OC_SA + NBLK],
                                    SA[:], axis=AX.X, op=ALU.add)
            for b0 in range(0, NBLK, 8):
                m2 = ps.tile([128, 4, 8, 16], f32, tag="m")
                for l in range(4):
                    nc.tensor.matmul(m2[:, l, :, :], W[l], child_view(b0, 8, 2, l),
                                     start=True, stop=True)
                nc.vector.tensor_tensor(
                    Dv[:, b0:b0 + 8, 24:88].rearrange("p b (n l) -> p b n l", l=4),
                    m2[:].transpose([0, 2, 3, 1]),
                    Db2[:, b0:b0 + 8, :, None].to_broadcast([128, 8, 16, 4]),
                    ALU.mult)
                # per-chunk endgame: S_b product + reductions
                nc.gpsimd.tensor_tensor(sbv[:, b0:b0 + 8, 4:88],
                                        Dv[:, b0:b0 + 8, 4:88],
                                        lgv[:, b0:b0 + 8, 4:88], ALU.mult)
                nc.vector.tensor_reduce(outp[:, OC_SB + b0:OC_SB + b0 + 8],
                                        sbv[:, b0:b0 + 8, 4:88],
                                        axis=AX.X, op=ALU.add)
                rhov = Dv[:, b0:b0 + 8, 4:88].rearrange("p b (n l) -> p b l n", l=4)
                nc.vector.tensor_reduce(
                    outp[:, OC_RHO + 4 * b0:OC_RHO + 4 * (b0 + 8)]
                    .rearrange("p (b l) -> p b l", l=4),
                    rhov, axis=AX.X, op=ALU.add)

            nc.sync.dma_start(out=o_out, in_=outp[:])

    nc.finalize()
    return nc


_NC_CACHE = {}


def _shard_inputs(t, a, b, pi, sp):
    tb, tbl, wv, cores, gids, host = _host_prep(t, a, b, pi, sp)
    in_maps = []
    for k in range(NCORES):
        in_maps.append({"ebd": cores[k][0], "lgbd": cores[k][1], "wv": wv})
    return in_maps, host


def kernel(t, t_limits, a, b, pi, sp):
    from concourse.bass_utils import run_bass_kernel_spmd
    if "nc" not in _NC_CACHE:
        _NC_CACHE["nc"] = build_bass()
    nc = _NC_CACHE["nc"]
    in_maps, host = _shard_inputs(t, a, b, pi, sp)
    res = run_bass_kernel_spmd(nc, in_maps, list(range(NCORES)))
    return _combine(res.results, host)


# revision 15
# speedup vs baseline: 1.1804x; 1.0158x over previous
"""Bass/Trainium2 kernel for nn_BottomUpHTMM (bottom-up hidden tree Markov model).

Tree: complete 4-ary, depth 7, 21845 nodes. G=16 models, C=8 states, 256 labels.

v2 design:
- Each core owns 32 independent depth-3 subtrees rooted at the 256 level-4
  nodes (85..340): 32*(1+4+16+64) = 2720 nodes/core. Partition dim = (g,c).
- The prior chain of the reference cancels algebraically
  (prior*beta_il == A@beta_children), so only beta is propagated.
- Downward pass is factorized: eps(n) = eps(root_b) * D(n) where
  D(child) = D(parent)*bnr(parent)*m(child), bnr = 1/(A@beta_ch),
  m(child) = (W_l @ beta)(child). D is independent of anything above the
  subtree root, so each core emits per-root partial sums
  S_b = sum D*logb, S_A = sum Db*q, S_rho[l] = sum_{slot l} D, plus the root
  betas. No collective: the host computes the 341-node tree top and contracts
  eps(root) with the S terms.
- Emissions come from one ap_gather per buffer out of a host-built 1280-entry
  table (cols 0..255 = sm_b; cols 256+256*pos+label = leaf beta pre-normalized
  with sm_pi folded in). logb likewise (leaf entries include log pi).
- Reciprocals are computed as Exp(-Ln(x)) on the Scalar engine (DVE reciprocal
  is ~8 cycles/elem).

Per-core column layout: 32 blocks * 88 cols; block = [3 pad | root | 4 L1 |
16 L2 | 64 leaves]. Children of block-col c level j at co[j+1]+4*p+l,
co = [3, 4, 8, 24].
"""

import numpy as np
import ml_dtypes

BF16 = ml_dtypes.bfloat16

G = 16
C = 8
M = 256
L = 4
NCORES = 8
T_SIZE = 21845
NBLK = 32                 # subtrees per core
BLK = 88                  # cols per block (3 pad + 85 nodes)
NCOL = NBLK * BLK         # 2816
CO = [3, 4, 8, 24]        # level col offsets within block
O21 = [0, 1, 5]           # level offsets within the 21 internal slots
NB = [1, 4, 16]           # parents per block per level
NIDX = NCOL // 16         # 176

# out cols
OC_RB = 0     # 0..32   root beta
OC_SB = 32    # 32..64  S_b
OC_SA = 64    # 64..96  S_A
OC_RHO = 96   # 96..224 S_rho (block-major x l)
NOUT = 224


def _softmax(x, axis):
    e = np.exp(x - x.max(axis=axis, keepdims=True))
    return e / e.sum(axis=axis, keepdims=True)


def _wrap_idx(idx):
    """idx j at partition j%16, slot j//16, replicated across 8 gpsimd cores."""
    idx = np.asarray(idx, dtype=np.int16)
    n = len(idx)
    assert n % 16 == 0
    grid = idx.reshape(n // 16, 16).T          # [16, n/16]
    return np.tile(grid, (8, 1))               # [128, n/16]


def _block_cols():
    """Per-block arrays: node id builder + (col -> level, pos-slot)."""
    # block-relative heap ids per col (0 for pads), using cols 3..88
    rel = np.full(BLK, -1, np.int64)
    rel[3] = 0
    for p in range(21):                        # internal block-rel heap ids 0..20
        for l in range(4):
            rel[CO[1] + 4 * p + l if p == 0 else 0] = 0
    # simpler: levels explicitly
    rel[3] = 0
    rel[4:8] = np.arange(1, 5)
    rel[8:24] = np.arange(5, 21)
    rel[24:88] = np.arange(21, 85)
    return rel


_REL = _block_cols()


def _host_prep(t, a, b, pi, sp):
    t = np.asarray(t)
    labels = t[:, 0].astype(np.int64)
    a = np.asarray(a, np.float64)
    b = np.asarray(b, np.float64)
    pi = np.asarray(pi, np.float64)
    sp = np.asarray(sp, np.float64)
    sm_a = _softmax(a, 1)
    sm_b = _softmax(b, 2)
    sm_pi = _softmax(pi, 1)
    sm_sp = _softmax(sp, 1)
    asp = sm_a * sm_sp[:, None, None, :]

    # tables [128, 1280]
    tb = np.zeros((128, 1280), np.float32)
    tbl = np.zeros((128, 1280), np.float32)
    sb128 = sm_b.reshape(128, M)
    tb[:, :M] = sb128
    tbl[:, :M] = np.log(sb128)
    for pos in range(4):
        v = sm_b * sm_pi[:, :, pos][:, :, None]          # [G,C,M]
        s = v.sum(1, keepdims=True)
        tb[:, M + M * pos:M + M * (pos + 1)] = (v / s).reshape(128, M)
        tbl[:, M + M * pos:M + M * (pos + 1)] = np.log(v).reshape(128, M)

    # weights [128, 1152]: W0..3 V0..3 mbd
    la = np.log(sm_a)
    wv = np.zeros((128, 1024), np.float32)
    for l in range(4):
        Wl = np.zeros((128, 128))
        Vl = np.zeros((128, 128))
        for g in range(G):
            Wl[g * C:(g + 1) * C, g * C:(g + 1) * C] = asp[g, :, :, l].T      # [j,i]
            Vl[g * C:(g + 1) * C, g * C:(g + 1) * C] = (asp * la)[g, :, :, l].T
        wv[:, 128 * l:128 * (l + 1)] = Wl
        wv[:, 512 + 128 * l:512 + 128 * (l + 1)] = Vl

    wv = wv.astype(BF16)

    # per-core node ids + gather idx
    # block-rel heap: node 0 root, children of p at 4p+1+l
    gid_rel = np.zeros(85, np.int64)
    cores = []
    gids = []
    for k in range(NCORES):
        idx = np.zeros(NCOL, np.int64)
        gid_all = np.zeros((NBLK, 85), np.int64)
        for bq in range(NBLK):
            root = 85 + NBLK * k + bq
            gid_rel[0] = root
            for p in range(21):
                for l in range(4):
                    gid_rel[4 * p + 1 + l] = 4 * gid_rel[p] + 1 + l
            gid_all[bq] = gid_rel
            base = BLK * bq
            lab = labels[gid_rel]
            idx[base + 3:base + 24] = lab[:21]                     # internal: sm_b
            pos = (gid_rel[21:] - 1) % 4
            idx[base + 24:base + 88] = M + M * pos + lab[21:]      # leaves
        ebd = tb[:, idx].astype(np.float32)
        lgbd = tbl[:, idx].astype(np.float32)
        for bq in range(NBLK):
            ebd[:, BLK * bq:BLK * bq + 3] = 1.0
            lgbd[:, BLK * bq:BLK * bq + 3] = 0.0
        cores.append((ebd.astype(BF16), lgbd.astype(BF16)))
        gids.append(gid_all)

    host = dict(labels=labels, asp=asp, sm_b=sm_b, sm_pi=sm_pi,
                log_a=la, log_b=np.log(sm_b), log_sp=np.log(sm_sp))
    return tb, tbl, wv, cores, gids, host


def _combine(results, host):
    """Host: 341-node tree top + contraction with per-core S terms."""
    labels = host["labels"]; asp = host["asp"]; sm_b = host["sm_b"]
    log_a = host["log_a"]; log_b = host["log_b"]; log_sp = host["log_sp"]

    beta = np.zeros((341, G, C))
    Ab = np.zeros((85, G, C))
    for k in range(NCORES):
        rb = np.asarray(results[k]["out"], np.float64)
        rbm = rb[:, OC_RB:OC_RB + NBLK].T.reshape(NBLK, G, C)
        beta[85 + NBLK * k:85 + NBLK * (k + 1)] = \
            rbm / rbm.sum(2, keepdims=True)
    for lev in range(3, -1, -1):
        s, e = (4 ** lev - 1) // 3, (4 ** (lev + 1) - 1) // 3
        ch = 4 * np.arange(s, e)[:, None] + np.arange(1, 5)[None, :]
        AbP = np.einsum('gijl,plgj->pgi', asp, beta[ch])
        tmp = np.einsum('gcp,pgc->pgc', sm_b[:, :, labels[s:e]], AbP)
        beta[s:e] = tmp / tmp.sum(2, keepdims=True)
        Ab[s:e] = AbP
    eps = np.zeros((341, G, C)); eps[0] = beta[0]
    a_lh = np.zeros(G); rho = np.zeros((G, L))
    for lev in range(0, 4):
        s, e = (4 ** lev - 1) // 3, (4 ** (lev + 1) - 1) // 3
        ch = 4 * np.arange(s, e)[:, None] + np.arange(1, 5)[None, :]
        pe = eps[s:e] / Ab[s:e]
        mch = np.einsum('gijl,plgj->pgil', asp, beta[ch])
        epsc = pe[:, :, :, None] * mch
        for l in range(4):
            eps[ch[:, l]] = epsc[:, :, :, l]
        rho += epsc.sum(2).sum(0)
        a_lh += np.einsum('pgi,gijl,gijl,plgj->g', pe, asp, log_a, beta[ch])
    b_lh = np.einsum('ugc,gcu->g', eps, log_b[:, :, labels[:341]])

    # device terms
    for k in range(NCORES):
        out = np.asarray(results[k]["out"], np.float64)
        er = eps[85 + NBLK * k:85 + NBLK * (k + 1)].reshape(NBLK, 128)  # [b,(g,c)]
        S_b = out[:, OC_SB:OC_SB + NBLK].T          # [b, 128]
        S_A = out[:, OC_SA:OC_SA + NBLK].T
        S_r = out[:, OC_RHO:OC_RHO + 4 * NBLK].T.reshape(NBLK, 4, 128)
        b_lh += (er * S_b).reshape(NBLK, G, C).sum(0).sum(1)
        a_lh += (er * S_A).reshape(NBLK, G, C).sum(0).sum(1)
        rho += np.einsum('blp,bp->pl', S_r, er).reshape(G, C, L).sum(1)
    sp_lh = (rho * log_sp).sum(1)
    return (a_lh + b_lh + sp_lh).astype(np.float32)


def build_bass():
    import concourse.bacc as bacc
    import concourse.tile as tile
    import concourse.mybir as mybir
    from concourse import bass

    f32 = mybir.dt.float32
    bf16 = mybir.dt.bfloat16
    AF = mybir.ActivationFunctionType
    ALU = mybir.AluOpType
    AX = mybir.AxisListType

    nc = bacc.Bacc("TRN2", target_bir_lowering=False, debug=False,
                   num_devices=NCORES)

    eb_in = nc.dram_tensor("ebd", [128, NCOL], bf16, kind="ExternalInput").ap()
    lgb_in = nc.dram_tensor("lgbd", [128, NCOL], bf16, kind="ExternalInput").ap()
    wv_in = nc.dram_tensor("wv", [128, 1024], bf16, kind="ExternalInput").ap()
    o_out = nc.dram_tensor("out", [128, NOUT], f32, kind="ExternalOutput").ap()

    with tile.TileContext(nc) as tc:
        with tc.tile_pool(name="per", bufs=1) as per, \
             tc.tile_pool(name="wrk", bufs=2) as wrk, \
             tc.tile_pool(name="ps", bufs=2, space="PSUM") as ps, \
             tc.tile_pool(name="ps3", bufs=3, space="PSUM") as ps3:

            wv = per.tile([128, 1024], bf16, tag="wv")
            W = [wv[:, 128 * l:128 * (l + 1)] for l in range(4)]
            V = [wv[:, 512 + 128 * l:512 + 128 * (l + 1)] for l in range(4)]

            eb = per.tile([128, NCOL], bf16, tag="eb")      # emission -> beta
            lgb = per.tile([128, NCOL], bf16, tag="lgb")    # log emission
            nc.sync.dma_start(out=wv[:], in_=wv_in)
            nc.scalar.dma_start(out=eb[:, :NCOL // 2], in_=eb_in[:, :NCOL // 2])
            nc.scalar.dma_start(out=eb[:, NCOL // 2:], in_=eb_in[:, NCOL // 2:])
            nc.sync.dma_start(out=lgb[:], in_=lgb_in)
            Dt = per.tile([128, NCOL], bf16, tag="Dt")      # eps factor D
            sbp = per.tile([128, NCOL], bf16, tag="sbp")    # D*logb scratch
            bnr = per.tile([128, NBLK, 21], f32, tag="bnr")
            SA = per.tile([128, NBLK, 21], f32, tag="SA")
            outp = per.tile([128, NOUT], f32, tag="outp")

            ebv = eb[:].rearrange("p (b c) -> p b c", b=NBLK)
            lgv = lgb[:].rearrange("p (b c) -> p b c", b=NBLK)
            Dv = Dt[:].rearrange("p (b c) -> p b c", b=NBLK)
            sbv = sbp[:].rearrange("p (b c) -> p b c", b=NBLK)

            def child_view(b0, nbl, j, l):
                """beta of l-th children of level-j parents: [128, nbl, NB[j]]"""
                v = ebv[:, b0:b0 + nbl, CO[j + 1]:CO[j + 1] + 4 * NB[j]]
                return v.rearrange("p b (n l) -> p b n l", l=4)[:, :, :, l]

            # ---------------- upward (no per-node normalization) ----------------
            for u, (j, b0, nbl) in enumerate([(2, 0, 16), (2, 16, 16),
                                              (1, 0, 32), (0, 0, 32)]):
                n_b = NB[j]
                ub = ps3.tile([128, nbl, n_b], f32, tag="ub")
                for l in range(4):
                    nc.tensor.matmul(ub[:], W[l], child_view(b0, nbl, j, l),
                                     start=(l == 0), stop=(l == 3))
                with nc.allow_low_precision(reason="bnr bf16, tol 2e-2"):
                    nc.vector.reciprocal_approx_fast(
                        bnr[:, b0:b0 + nbl, O21[j]:O21[j] + n_b], ub[:])
                nc.vector.tensor_tensor(
                    ebv[:, b0:b0 + nbl, CO[j]:CO[j] + n_b],
                    ebv[:, b0:b0 + nbl, CO[j]:CO[j] + n_b], ub[:], ALU.mult)

            # root (unnormalized) betas to output
            nc.scalar.copy(out=outp[:, OC_RB:OC_RB + NBLK], in_=ebv[:, :, 3])

            # ---------------- downward (D chain + S terms) ----------------
            Db0 = bnr[:, :, 0]                                   # [128, 32]
            # j = 0
            m0 = ps.tile([128, 4, NBLK], f32, tag="m")
            for l in range(4):
                nc.tensor.matmul(m0[:, l, :], W[l],
                                 child_view(0, NBLK, 0, l)
                                 .rearrange("p b n -> p (b n)"),
                                 start=True, stop=True)
            q0 = ps.tile([128, NBLK], f32, tag="q")
            for l in range(4):
                nc.tensor.matmul(q0[:], V[l],
                                 child_view(0, NBLK, 0, l)
                                 .rearrange("p b n -> p (b n)"),
                                 start=(l == 0), stop=(l == 3))
            nc.vector.tensor_tensor(Dv[:, :, 4:8], m0[:].transpose([0, 2, 1]),
                                    Db0[:, :, None].to_broadcast([128, NBLK, 4]),
                                    ALU.mult)
            nc.vector.tensor_tensor(SA[:, :, 0:1], Db0[:, :, None], q0[:, :, None],
                                    ALU.mult)
            # j = 1
            Db1 = wrk.tile([128, NBLK, 4], bf16, tag="db1")
            nc.vector.tensor_tensor(Db1[:], Dv[:, :, 4:8], bnr[:, :, 1:5], ALU.mult)
            m1 = ps.tile([128, 4, NBLK, 4], f32, tag="m")
            for l in range(4):
                nc.tensor.matmul(m1[:, l, :, :], W[l], child_view(0, NBLK, 1, l),
                                 start=True, stop=True)
            q1 = ps.tile([128, NBLK, 4], f32, tag="q")
            for l in range(4):
                nc.tensor.matmul(q1[:], V[l], child_view(0, NBLK, 1, l),
                                 start=(l == 0), stop=(l == 3))
            nc.vector.tensor_tensor(
                Dv[:, :, 8:24].rearrange("p b (n l) -> p b n l", l=4),
                m1[:].transpose([0, 2, 3, 1]),
                Db1[:, :, :, None].to_broadcast([128, NBLK, 4, 4]), ALU.mult)
            nc.vector.tensor_tensor(SA[:, :, 1:5], Db1[:], q1[:], ALU.mult)
            # j = 2
            Db2 = wrk.tile([128, NBLK, 16], bf16, tag="db2")
            nc.vector.tensor_tensor(Db2[:], Dv[:, :, 8:24], bnr[:, :, 5:21],
                                    ALU.mult)
            q2 = ps.tile([128, NBLK, 16], f32, tag="q")
            for l in range(4):
                nc.tensor.matmul(q2[:], V[l], child_view(0, NBLK, 2, l),
                                 start=(l == 0), stop=(l == 3))
            nc.vector.tensor_tensor(SA[:, :, 5:21], Db2[:], q2[:], ALU.mult)
            nc.vector.tensor_reduce(outp[:, OC_SA:OC_SA + NBLK],
                                    SA[:], axis=AX.X, op=ALU.add)
            for b0 in range(0, NBLK, 8):
                m2 = ps.tile([128, 4, 8, 16], f32, tag="m")
                for l in range(4):
                    nc.tensor.matmul(m2[:, l, :, :], W[l], child_view(b0, 8, 2, l),
                                     start=True, stop=True)
                nc.vector.tensor_tensor(
                    Dv[:, b0:b0 + 8, 24:88].rearrange("p b (n l) -> p b n l", l=4),
                    m2[:].transpose([0, 2, 3, 1]),
                    Db2[:, b0:b0 + 8, :, None].to_broadcast([128, 8, 16, 4]),
                    ALU.mult)
                # per-chunk endgame: S_b product + reductions
                nc.gpsimd.tensor_tensor(sbv[:, b0:b0 + 8, 4:88],
                                        Dv[:, b0:b0 + 8, 4:88],
                                        lgv[:, b0:b0 + 8, 4:88], ALU.mult)
                nc.vector.tensor_reduce(outp[:, OC_SB + b0:OC_SB + b0 + 8],
                                        sbv[:, b0:b0 + 8, 4:88],
                                        axis=AX.X, op=ALU.add)
                rhov = Dv[:, b0:b0 + 8, 4:88].rearrange("p b (n l) -> p b l n", l=4)
                nc.vector.tensor_reduce(
                    outp[:, OC_RHO + 4 * b0:OC_RHO + 4 * (b0 + 8)]
                    .rearrange("p (b l) -> p b l", l=4),
                    rhov, axis=AX.X, op=ALU.add)

            nc.sync.dma_start(out=o_out, in_=outp[:])

    nc.finalize()
    return nc


_NC_CACHE = {}


def _shard_inputs(t, a, b, pi, sp):
    tb, tbl, wv, cores, gids, host = _host_prep(t, a, b, pi, sp)
    in_maps = []
    for k in range(NCORES):
        in_maps.append({"ebd": cores[k][0], "lgbd": cores[k][1], "wv": wv})
    return in_maps, host


def kernel(t, t_limits, a, b, pi, sp):
    from concourse.bass_utils import run_bass_kernel_spmd
    if "nc" not in _NC_CACHE:
        _NC_CACHE["nc"] = build_bass()
    nc = _NC_CACHE["nc"]
    in_maps, host = _shard_inputs(t, a, b, pi, sp)
    res = run_bass_kernel_spmd(nc, in_maps, list(range(NCORES)))
    return _combine(res.results, host)
